# revision 13
# baseline (speedup 1.0000x reference)
"""Bass/Trainium2 kernel for nn_ABAgInteractionLayer (cross-attention + residual).

Sharding: data-parallel over batch B=8 -> one batch element per NeuronCore.
No collectives; each core computes its full batch slice.

Algebraic refactoring (host-side, weight-only constant folding):
  scores = (Xb Wq + bq)(Xg Wk + bk)^T / 16
         = Xb M Xg^T + (r 1^T) Xg^T + const_per_q      M = Wq Wk^T/16,
                                                       r = bq^T Wk^T/16
  (the const-per-q term from bk cancels in softmax)
  inter  = (P (Xg Wv + bv)) Wo + bo = P Xg N + (bv Wo + bo)   N = Wv Wo
  (P rows sum to 1, so bv contributes a constant vector folded into res)
So the device kernel never computes K or V projections: attention runs
directly against raw Xg (e4m3 feature-major for scores, e5m2 token-major
for AV), and only two 256x256 projections remain (M on the way in, N on
the way out).

Per-core pipeline (one batch b):
  q8 = e4m3(16 * (Xb_bf16 @ M_bf16 + r))              [feature-major]
  sT[k,q] = k8 . q8      (e4m3 DoubleRow, [k,q] layout so exp(sT) is the
                          stationary operand of the AV matmul)
  eT = exp(sT/16)  as fp8e5m2, via two engines:
       ACT: native Exp activation (scale=1/16), pair-fused [128,1024]
       DVE: one-pass Schraudolph: e5m2 bits are the top 8 bits of fp16,
            so int8(x*(4/ln2)/16 + 60) bitcast to e5m2 IS exp(x/16).
            The constant multiplicative bias cancels in the softmax ratio.
  AV[q,:] = sum_k eT[k,q] * vab[k,:]   (e5m2 DoubleRow; vab = [Xg | 1 | pad],
                                        col 256 accumulates Z = sum_k eT)
  out[q,:] = (AV[:,0:256]/Z) @ N + (Xb + bv Wo + bo)  (bf16 out-projection)

Scheduling: scores+exp for q-block n+1 are interleaved matmul-by-matmul
with the AV chains of q-block n, so the exp engines (the bottleneck) never
starve while PE streams AV. Scores land in a 4-bank PSUM ring; exp
consumes pairs of banks.
"""

import sys

if "/opt/trn_rl_repo" not in sys.path:
    sys.path.insert(0, "/opt/trn_rl_repo")

import ml_dtypes
import numpy as np

import concourse.bacc as bacc
import concourse.bass as bass
import concourse.mybir as mybir
import concourse.tile as tile
from concourse import masks
from concourse.bass_utils import run_bass_kernel_spmd

B, L, A, F = 8, 512, 5, 256
H = 256
LQ = L * A          # 2560 query tokens
LK = 1024 * 5       # 5120 key tokens
NCORES = 8
QBLK = 512
NQB = LQ // QBLK    # 5
KT = 128
NKT = LK // KT      # 40
NKP = NKT // 2      # 20 k-tile pairs
VW = 272            # Xg | ones | pad (fp8 DoubleRow pair step must be %16)
DT = mybir.dt.float32
BF = mybir.dt.bfloat16
F8E4 = mybir.dt.float8e4
F8E5 = mybir.dt.float8e5
I8 = mybir.dt.int8
NP_BF = ml_dtypes.bfloat16
NP_E4 = ml_dtypes.float8_e4m3
NP_E5 = ml_dtypes.float8_e5m2

QSCALE = 16.0       # q8 = e4m3(16*q''); exp compensates with scale=1/16
A_SC = (4.0 / np.log(2.0)) / QSCALE
B_SC = 60.0         # e5m2 exponent bias (15*4); DVE converts round-to-nearest

# exp engine assignment per k-tile pair: 'A'=ACT native exp, 'D'=DVE
# one-pass Schraudolph.  Runs (AAD) rather than strict alternation: the
# scores ring holds only 2 pairs, so a strictly alternating engine waits
# on ITS OWN previous exp via the ring (ping-pong latency ~2us/pair);
# runs let each engine's serial work hide the recycle latency.
# 14 A / 6 D: the whole epilogue lives on DVE.
ASSIGN = "AADAADAADAADAADAADAD"
assert len(ASSIGN) == NKP and ASSIGN.count("A") == 13
# q-block 0 runs with no AV work to overlap, so balance exp 50/50 by time
ASSIGN0 = "AADAD" * 4
# M-projection epilogue (PSUM->q8 convert) engine per tile (10 tiles)
MP_ASSIGN = "DADADADADA"

RES_VIA_PSUM = False  # DMA cannot touch PSUM on TRN2 (bass asserts
                      # SBUF/DRAM only), so residual rides the final
                      # scalar_tensor_tensor instead


def build():
    nc = bacc.Bacc("TRN2", target_bir_lowering=False, debug=False,
                   num_devices=NCORES)
    xbT = nc.dram_tensor("xbT", [128, 2, LQ], BF, kind="ExternalInput")
    k8d = nc.dram_tensor("k8d", [128, 2, LK], F8E4, kind="ExternalInput")
    vabd = nc.dram_tensor("vabd", [128, NKP, 2, VW], F8E5,
                          kind="ExternalInput")
    res = nc.dram_tensor("res", [LQ, H], DT, kind="ExternalInput")
    md = nc.dram_tensor("md", [128, 2, H], BF, kind="ExternalInput")
    nd = nc.dram_tensor("nd", [128, 2, H], BF, kind="ExternalInput")
    rqd = nc.dram_tensor("rqd", [128, 2], DT, kind="ExternalInput")
    out = nc.dram_tensor("out", [LQ, H], DT, kind="ExternalOutput")

    ActF = mybir.ActivationFunctionType
    Alu = mybir.AluOpType

    with tile.TileContext(nc) as tc:
        with (
            tc.tile_pool(name="const", bufs=1) as cp,
            tc.tile_pool(name="persist", bufs=1) as pp,
            tc.tile_pool(name="sps", bufs=3,
                         space=bass.MemorySpace.PSUM) as spp,
            tc.tile_pool(name="avps", bufs=2,
                         space=bass.MemorySpace.PSUM) as avp,
            tc.tile_pool(name="exbufs", bufs=2 * NKP) as exp_pool,
            tc.tile_pool(name="epil", bufs=2) as elp,
        ):
            m_s = cp.tile([128, 2, H], BF, tag="m")
            n_s = cp.tile([128, 2, H], BF, tag="n")
            rq_s = cp.tile([128, 2], DT, tag="rq")
            nc.sync.dma_start(m_s[:], md[:])
            nc.sync.dma_start(rq_s[:], rqd[:])
            zb = cp.tile([128, 1], DT, tag="zb")
            nc.vector.memset(zb[:], 0.0)
            ident = cp.tile([128, 128], BF, tag="ident")
            masks.make_identity(nc, ident[:])

            k8 = pp.tile([128, 2, LK], F8E4, tag="k8")
            vab = pp.tile([128, NKP, 2, VW], F8E5, tag="vab")
            # per-block tiles (separate Tile objects keep dependency
            # tracking exact regardless of subtile granularity)
            xb_b = [pp.tile([128, 2, QBLK], BF, tag=f"xb{t}", name=f"xb{t}")
                    for t in range(NQB)]
            q8_b = [pp.tile([128, 2, QBLK], F8E4, tag=f"q8{t}", name=f"q8{t}")
                    for t in range(NQB)]

            nc.sync.dma_start(xb_b[0][:], xbT[:, :, 0:QBLK])
            for h in range(2):
                s0, s1 = h * LK // 2, (h + 1) * LK // 2
                nc.sync.dma_start(k8[:, :, s0:s1], k8d[:, :, s0:s1])
            for t in range(1, NQB):
                nc.sync.dma_start(xb_b[t][:], xbT[:, :, t * QBLK:
                                                  (t + 1) * QBLK])
            nc.sync.dma_start(n_s[:], nd[:])
            for h in range(2):
                p0, p1 = h * NKP // 2, (h + 1) * NKP // 2
                nc.sync.dma_start(vab[:, p0:p1, :, :], vabd[:, p0:p1, :, :])

            def mproj_tile(i):
                # q8[t0][:, co, :] = e4m3(16*(M[:,co].T @ XbT) + 16*r)
                t0, co = i // 2, i % 2
                sp = spp.tile([128, 2, QBLK], DT, tag="sp", name="sp")
                ps = sp[:, 0, :]
                for ci in range(2):
                    nc.tensor.matmul(
                        ps, m_s[:, ci, co * 128:(co + 1) * 128],
                        xb_b[t0][:, ci, :],
                        start=(ci == 0), stop=(ci == 1))
                dst = q8_b[t0][:, co, :]
                if MP_ASSIGN[i] == "A":
                    nc.scalar.activation(dst, ps, ActF.Identity,
                                         bias=rq_s[:, co:co + 1],
                                         scale=QSCALE)
                else:
                    nc.vector.tensor_scalar(dst, ps, QSCALE,
                                            rq_s[:, co:co + 1],
                                            Alu.mult, Alu.add)

            def scores_pair(p, qb, exb_list, assign):
                sp = spp.tile([128, 2, QBLK], DT, tag="sp", name="sp")
                for half in range(2):
                    t = 2 * p + half
                    nc.tensor.matmul(
                        sp[:, half, :],
                        k8[:, :, t * KT:(t + 1) * KT],
                        q8_b[qb][:, :, :],
                        perf_mode=mybir.MatmulPerfMode.DoubleRow,
                        start=True, stop=True)
                exbt = exp_pool.tile([128, 2, QBLK], I8, tag="exb",
                                     name="exb")
                exb_list[p] = exbt
                if assign[p] == "A":
                    nc.scalar.activation(exbt[:].bitcast(F8E5), sp[:],
                                         ActF.Exp, bias=zb[:],
                                         scale=1.0 / QSCALE)
                else:
                    nc.vector.tensor_scalar(exbt[:], sp[:], A_SC, B_SC,
                                            Alu.mult, Alu.add)

            def av_mms(j, kps, exb_list, av):
                for kp in kps:
                    nc.tensor.matmul(
                        av,
                        exb_list[kp][:, :, j * 128:(j + 1) * 128]
                        .bitcast(F8E5),
                        vab[:, kp, :, :],
                        perf_mode=mybir.MatmulPerfMode.DoubleRow,
                        start=(kp == 0), stop=(kp == NKP - 1))

            # Epilogue stages are scheduled 2 steps apart so each stage's
            # cross-engine inputs are produced >1us earlier: neither PE nor
            # DVE blocks at its queue head on a fresh dependency.
            def epi_s0(j, qb, av, st):
                rows = qb * QBLK + j * 128
                rec = elp.tile([128, 1], DT, tag="rec")
                nc.vector.reciprocal(rec[:], av[:, H:H + 1])
                avn = elp.tile([128, H], BF, tag="avn")
                nc.vector.tensor_scalar_mul(avn[:], av[:, 0:H], rec[:])
                res_t = elp.tile([128, H], DT, tag="res")
                nc.sync.dma_start(res_t[:], res[rows:rows + 128, :])
                st["avn"], st["res_t"] = avn, res_t

            def epi_s1(j, qb, st):
                # op + transpose scratch live in one scores-pool slot
                # (bank 0 of it): op = [:,0,0:256], tp_c = [:,0,256+64c:...]
                avn = st["avn"]
                ep = spp.tile([128, 2, QBLK], DT, tag="sp", name="sp")
                avnT = elp.tile([128, 2, 128], BF, tag="avnT")
                for c in range(2):
                    tp = ep[:, 0, 256 + 64 * c:320 + 64 * c].bitcast(BF)
                    nc.tensor.transpose(tp, avn[:, c * 128:(c + 1) * 128],
                                        ident[:])
                for c in range(2):
                    tp = ep[:, 0, 256 + 64 * c:320 + 64 * c].bitcast(BF)
                    nc.vector.tensor_copy(avnT[:, c, :], tp)
                st["ep"], st["avnT"] = ep, avnT

            def epi_s2(j, qb, st):
                rows = qb * QBLK + j * 128
                avnT, ep = st["avnT"], st["ep"]
                op = ep[:, 0, 0:H]
                for c in range(2):
                    nc.tensor.matmul(op, avnT[:, c, :], n_s[:, c, :],
                                     start=(c == 0), stop=(c == 1))
                out_t = elp.tile([128, H], DT, tag="out")
                nc.vector.scalar_tensor_tensor(out_t[:], op, 1.0,
                                               st["res_t"], Alu.mult, Alu.add)
                nc.sync.dma_start(out[rows:rows + 128, :], out_t[:])

            # ---- phase 1: M-projection + scores/exp for q-block 0 ----
            exb_cur = [None] * NKP
            for i in range(4):
                mproj_tile(i)
            for p in range(NKP):
                scores_pair(p, 0, exb_cur, ASSIGN0)
                if p < 6:
                    mproj_tile(4 + p)

            # ---- main schedule: flat global-step event list ----
            # step g in qblock qb covers: scores pair p of qb+1 (g%20),
            # AV chains of qb shifted +2 steps, epilogue stages +6/+8/+10.
            events = []  # (g, prio, fn)
            exb_all = [exb_cur] + [[None] * NKP for _ in range(NQB - 1)]
            for qb in range(NQB):
                g0 = qb * NKP
                if qb + 1 < NQB:
                    for p in range(NKP):
                        events.append((g0 + p, 0,
                                       lambda p=p, qb=qb:
                                       scores_pair(p, qb + 1,
                                                   exb_all[qb + 1], ASSIGN)))
                avs = {}
                for j in range(4):
                    for s in range(5):
                        def av_step(j=j, s=s, qb=qb, avs=avs):
                            if s == 0:
                                avs[j] = avp.tile([128, VW], DT, tag="av",
                                                  name="av")
                            av_mms(j, range(s * 4, s * 4 + 4),
                                   exb_all[qb], avs[j])
                        events.append((g0 + 5 * j + 2 + s, 1, av_step))
                    st = {}
                    events.append((g0 + 5 * j + 6, 2,
                                   lambda j=j, qb=qb, st=st, avs=avs:
                                   epi_s0(j, qb, avs[j], st)))
                    events.append((g0 + 5 * j + 8, 2,
                                   lambda j=j, qb=qb, st=st:
                                   epi_s1(j, qb, st)))
                    events.append((g0 + 5 * j + 10, 2,
                                   lambda j=j, qb=qb, st=st:
                                   epi_s2(j, qb, st)))
            events.sort(key=lambda e: (e[0], e[1]))
            for _, _, fn in events:
                fn()

    nc.compile()
    return nc


_nc_cache = None
last_results = None


def _get_nc():
    global _nc_cache
    if _nc_cache is None:
        _nc_cache = build()
    return _nc_cache


def kernel(**inputs):
    global last_results
    ab = np.ascontiguousarray(inputs["ab"], dtype=np.float32)
    ag = np.ascontiguousarray(inputs["ag"], dtype=np.float32)
    Wq = np.asarray(inputs["Wq"], dtype=np.float32)
    Wk = np.asarray(inputs["Wk"], dtype=np.float32)
    Wv = np.asarray(inputs["Wv"], dtype=np.float32)
    Wo = np.asarray(inputs["Wo"], dtype=np.float32)
    bq = np.asarray(inputs["bq"], dtype=np.float32)
    bk = np.asarray(inputs["bk"], dtype=np.float32)
    bv = np.asarray(inputs["bv"], dtype=np.float32)
    bo = np.asarray(inputs["bo"], dtype=np.float32)

    s = np.float32(1.0 / np.sqrt(np.float32(H)))
    M = (s * (Wq @ Wk.T)).astype(NP_BF)          # [256 f, 256 d]
    N = (Wv @ Wo).astype(NP_BF)                  # [256 d, 256 f]
    r = s * (bq @ Wk.T)                          # [256 d]; bk cancels
    cvec = bv @ Wo + bo                          # [256 f]

    def fmajor(w, np_dt):
        # [256, X] -> [128, 2, X] with row c*128+p -> [p, c]
        return np.ascontiguousarray(
            w.reshape(2, 128, -1).transpose(1, 0, 2).astype(np_dt))

    md = fmajor(M, NP_BF)
    nd = fmajor(N, NP_BF)
    rqd = np.ascontiguousarray((QSCALE * r).reshape(2, 128).T,
                               dtype=np.float32)

    in_maps = []
    for b in range(B):
        xb = ab[b].reshape(LQ, F)
        xg = ag[b].reshape(LK, F)
        in_maps.append({
            "xbT": fmajor(xb.T, NP_BF),
            "k8d": fmajor(xg.T, NP_E4),
            "vabd": _vab_layout(xg),
            "res": np.ascontiguousarray(xb + cvec[None, :]),
            "md": md, "nd": nd, "rqd": rqd,
        })

    nc = _get_nc()
    last_exc = None
    for _attempt in range(3):
        try:
            last_results = run_bass_kernel_spmd(
                nc, in_maps, core_ids=list(range(NCORES)))
            break
        except Exception as e:  # transient device flakes
            last_exc = e
    else:
        raise last_exc
    return np.stack([last_results.results[b]["out"].reshape(L, A, F)
                     for b in range(B)]).astype(np.float32)


def _vab_layout(xg):
    # [5120, 256] -> [128, NKP, 2, VW] e5m2: token t = (kp*2+c)*128+p,
    # cols 0:256 = xg row, col 256 = 1.0, rest 0.
    v = np.zeros((NKP, 2, 128, VW), dtype=NP_E5)
    v[:, :, :, 0:F] = xg.reshape(NKP, 2, 128, F).astype(NP_E5)
    v[:, :, :, F] = np.float32(1.0)
    return np.ascontiguousarray(v.transpose(2, 0, 1, 3))
